# revision 1
# baseline (speedup 1.0000x reference)
"""Distributed Trainium2 kernel for quantized-mixed int8 matmul dequant.

Reference computation (M = K = N = 4096):
    xf = (x - X_ZP) * X_SCALE      # x int32 values in [-128, 127]
    yf = (y - Y_ZP) * Y_SCALE      # y int32 values in [0, 255]
    out = xf @ yf                  # float32 [M, N]

Strategy: 2D-shard the GEMM over 8 NeuronCores as a 2x4 grid
(M split 2 ways, N split 4 ways -> per-core C tile of 2048 x 1024),
with the matmul run in fp8 (E4M3) DoubleRow mode (double-pumped PE:
two k-rows per cell per pass, 2x bf16 matmul throughput; measured
216ns per [256k x 128m x 512n] matmul = the fp8 roofline, 110.6us
of matmul work per core).

fp8 precision scheme (rel err ~7e-3 vs the 2e-2 gate):
  x is centered:  x~ = (x + 0.5) * sqrt(S)   in [-127.5, 127.5]*sqrt(S)
  y is shifted:   y~ = (y - 160) * sqrt(S)   in [-160, 95]*sqrt(S)
  out[m,n] = sum_k x~ y~ + g[n],  g[n] = 65.5 * S * colsum(y - 160)[n]
The sqrt(S) prescale (S = X_SCALE*Y_SCALE) keeps fp8 relative precision
identical while making PSUM hold final-scale values, so the epilogue is
a single tensor_add of the exact (host-computed, fp32) g correction.
Centering x halves its top-end quantization step (128..193 would round
at step 16; +-127.5 rounds at step 8). fp8 e4m3 products are exact in
the PE's e10m10 intermediate, so a host numpy simulation of the fp8
rounding predicts the HW result bit-for-bit (verified: 6.991e-03 both).

Both fp8 operand shards are SBUF-resident (x 8MB + y 4MB of ~26MB), so
after the upload ramp the PE runs with zero DMA waits. Upload layout is
chosen so the ramp critical path is short: the first m-tile sweep needs
ALL of y but only the first m-half of x, so y is the SMALL shard (4MB,
~20us on one ring) and x's two m-halves upload in order on the other
ring. Each ring carries one sequential HBM stream — interleaving x/y
chunks across both rings makes 4 concurrent HBM streams, halves upload
bandwidth, stalls the PE >5us, and the HAM clock gate then runs the PE
~1.2x slow for the WHOLE kernel (216 -> 259ns per matmul, measured).
Per-double-chunk DMAs (256KB) keep any individual PE wait well under
the ~3.4us HAM window during the ramp.

Loop order is m-tile outer, k inner, n-group innermost: each
stationary x-slice is reused across the 2 moving n-groups and
LDWEIGHTS (135ns, no FWL in DoubleRow mode) hides under the 216ns
matmuls.
"""

import numpy as np
import ml_dtypes

import concourse.bacc as bacc
import concourse.mybir as mybir
import concourse.tile as tile
from concourse.bass_utils import run_bass_kernel_spmd

M = K = N = 4096
X_SCALE, X_ZP = 0.03, -66
Y_SCALE, Y_ZP = 0.025, 160
S = X_SCALE * Y_SCALE
SQS = np.float32(np.sqrt(S))
CX = 65.5                 # x centering shift: x - X_ZP = (x + 0.5) + CX

NCORES = 8
MSPLIT, NSPLIT = 2, 4
MC = M // MSPLIT          # 2048 rows of C per core
NC = N // NSPLIT          # 1024 cols of C per core
P = 128                   # partitions
KC2 = K // (2 * P)        # 16 double-chunks (256 k-rows each)
MT = MC // P              # 16 m-tiles
MH = 2                    # x uploaded in MH m-halves (first sweep needs one)
MCH = MC // MH            # 1024 x-columns per half
NF = 512                  # matmul out free dim (one PSUM bank at fp32)
NG = NC // NF             # 2 n-groups
XQ = 1                    # k-double-chunks per x tile (256KB DMAs)
# y tile sizes in double-chunks (256KB units). The head stays uniform
# 256KB — every variation that batched or mixed the EARLY upload
# measured the PE at ~2.0GHz (259ns/matmul) for the whole kernel
# instead of 2.4GHz (216ns). Batching only the tail cuts trigger
# overhead where it is safe.
YSIZES = [1] * 10 + [2] * 3
YOFFS = np.cumsum([0] + YSIZES[:-1]).tolist()
assert sum(YSIZES) == KC2

FP8 = mybir.dt.float8e4
E4NP = ml_dtypes.float8_e4m3

WARM_MM = 12              # 12 dummy warmup matmuls held 2.4GHz in 15/15
                          # runs; 8 lost the clock lottery in 4/8 runs
LAST_SPLIT = True

_CACHE = {}


def _build():
    nc = bacc.Bacc("TRN2", target_bir_lowering=False, debug=False)
    xt = nc.dram_tensor("xt", [MH, KC2 // XQ, P, 2 * XQ, MCH], FP8,
                        kind="ExternalInput")
    yts = [nc.dram_tensor(f"y{q}", [P, 2 * sz, NC], FP8, kind="ExternalInput")
           for q, sz in enumerate(YSIZES)]
    g = nc.dram_tensor("g", [P, NC], mybir.dt.float32, kind="ExternalInput")
    out = nc.dram_tensor("out", [MT, NG, P, NF], mybir.dt.float32,
                         kind="ExternalOutput")

    with tile.TileContext(nc) as tc:
        with (
            tc.tile_pool(name="warm_pool", bufs=1) as warm_pool,
            tc.tile_pool(name="xb_pool", bufs=MH * KC2 // XQ) as xb_pool,
            tc.tile_pool(name="yb_pool", bufs=len(YSIZES)) as yb_pool,
            tc.tile_pool(name="g_pool", bufs=1) as g_pool,
            tc.tile_pool(name="ot_pool", bufs=8) as ot_pool,
            tc.tile_pool(name="ps_pool", bufs=8, space="PSUM") as ps_pool,
        ):
            # PE warm-up: the upload ramp is DMA-heavy with little matmul
            # work; burn dummy matmuls so the HAM clock gate un-throttles
            # before the real matmuls issue.
            wt = warm_pool.tile([P, NF], mybir.dt.bfloat16, tag="wt")
            nc.vector.memset(wt[:], 0.0)
            wps = ps_pool.tile([64, NF], mybir.dt.float32, tag="ps", name="wps")
            for _ in range(WARM_MM):
                nc.tensor.matmul(wps[:], wt[:, :64], wt[:], start=True, stop=True)

            gt = g_pool.tile([P, NC], mybir.dt.float32, tag="gt")
            nc.gpsimd.dma_start(out=gt[:], in_=g[:, :])
            xb = [[xb_pool.tile([P, 2 * XQ, MCH], FP8, tag="xb",
                                name=f"xb{h}_{q}")
                   for q in range(KC2 // XQ)] for h in range(MH)]
            yb = [yb_pool.tile([P, 2 * sz, NC], FP8, tag="yb", name=f"yb{q}")
                  for q, sz in enumerate(YSIZES)]
            for q in range(len(YSIZES)):
                nc.scalar.dma_start(out=yb[q][:], in_=yts[q][:, :, :])
            for h in range(MH):
                for q in range(KC2 // XQ):
                    nc.sync.dma_start(out=xb[h][q][:], in_=xt[h, q, :, :, :])
            # k -> (y tile, chunk offset within tile)
            ymap = {}
            for q, (sz, off) in enumerate(zip(YSIZES, YOFFS)):
                for c in range(sz):
                    ymap[off + c] = (q, c)

            out_dma_engines = [nc.sync, nc.scalar]
            nout = [0]

            def drain(m, gi, psum):
                ot = ot_pool.tile([P, NF], mybir.dt.float32, tag="ot",
                                  name=f"ot{m}_{gi}")
                nc.vector.tensor_add(ot[:], psum[:],
                                     gt[:, gi * NF:(gi + 1) * NF])
                eng = out_dma_engines[nout[0] % 2]
                nout[0] += 1
                eng.dma_start(out=out[m, gi, :, :], in_=ot[:])

            def mm_sweep(m, groups, psums):
                h, mm = divmod(m, MT // MH)
                for k in range(KC2):
                    q, c = divmod(k, XQ)
                    xs = xb[h][q][:, 2 * c:2 * c + 2, mm * P:(mm + 1) * P]
                    yq, yc = ymap[k]
                    for j, gi in enumerate(groups):
                        nc.tensor.matmul(
                            psums[j][:], xs,
                            yb[yq][:, 2 * yc:2 * yc + 2,
                                   gi * NF:(gi + 1) * NF],
                            start=(k == 0), stop=(k == KC2 - 1),
                            perf_mode=mybir.MatmulPerfMode.DoubleRow)

            def ps_tiles(m, groups):
                return [ps_pool.tile([P, NF], mybir.dt.float32,
                                     tag="ps", name=f"ps{m}_{gi}")
                        for gi in groups]

            n_plain = MT - 1 if LAST_SPLIT else MT
            for m in range(n_plain):
                groups = list(range(NG))
                psums = ps_tiles(m, groups)
                mm_sweep(m, groups, psums)
                for gi in groups:
                    drain(m, gi, psums[gi])

            if LAST_SPLIT:
                m = MT - 1
                for gi in range(NG):
                    psums = ps_tiles(m, [gi])
                    mm_sweep(m, [gi], psums)
                    drain(m, gi, psums[0])
    nc.compile()
    return nc


def _get_nc():
    if "nc" not in _CACHE:
        _CACHE["nc"] = _build()
    return _CACHE["nc"]


def _chunk_block(a2d, ncols):
    """[K, ncols] -> [KC2, P, 2, ncols] with (c, p, i) -> k = 256c + 128i + p
    (the DoubleRow pairing)."""
    return np.ascontiguousarray(
        a2d.reshape(KC2, 2, P, ncols).transpose(0, 2, 1, 3))


def _shard(x, y):
    x = np.asarray(x, dtype=np.int32)
    y = np.asarray(y, dtype=np.int32)
    xq = ((x.astype(np.float32) + np.float32(0.5)) * SQS).astype(E4NP)
    yq = ((y.astype(np.float32) - np.float32(160.0)) * SQS).astype(E4NP)
    # exact column correction for the x centering shift
    gfull = (CX * S) * (y.astype(np.float64).sum(axis=0) - 160.0 * K)
    gfull = gfull.astype(np.float32)

    xts = []
    for mi in range(MSPLIT):
        blk = _chunk_block(
            np.ascontiguousarray(xq[mi * MC:(mi + 1) * MC, :].T), MC)
        # m-halves + quad-chunk tiles: [MH, KC2/XQ, P, 2*XQ, MCH]
        xts.append(np.ascontiguousarray(
            blk.reshape(KC2 // XQ, XQ, P, 2, MH, MCH)
            .transpose(4, 0, 2, 1, 3, 5)
            .reshape(MH, KC2 // XQ, P, 2 * XQ, MCH)))
    ys = []
    for ni in range(NSPLIT):
        blk = _chunk_block(
            np.ascontiguousarray(yq[:, ni * NC:(ni + 1) * NC]), NC)
        tiles = {}
        for q, (sz, off) in enumerate(zip(YSIZES, YOFFS)):
            # [sz, P, 2, NC] -> [P, 2*sz, NC]
            tiles[f"y{q}"] = np.ascontiguousarray(
                blk[off:off + sz].transpose(1, 0, 2, 3)
                .reshape(P, 2 * sz, NC))
        ys.append(tiles)
    gs = [np.ascontiguousarray(
              np.broadcast_to(gfull[ni * NC:(ni + 1) * NC], (P, NC)))
          for ni in range(NSPLIT)]
    in_maps = []
    for c in range(NCORES):
        mi, ni = divmod(c, NSPLIT)
        in_maps.append({"xt": xts[mi], "g": gs[ni], **ys[ni]})
    return in_maps


def _gather(results):
    out = np.empty((M, N), dtype=np.float32)
    for c in range(NCORES):
        mi, ni = divmod(c, NSPLIT)
        blk = results[c]["out"]  # [MT, NG, P, NF]
        out[mi * MC:(mi + 1) * MC, ni * NC:(ni + 1) * NC] = \
            blk.transpose(0, 2, 1, 3).reshape(MC, NC)
    return out


def run(x, y, **spmd_kwargs):
    """Run and return (full_output, BassKernelResults)."""
    nc = _get_nc()
    in_maps = _shard(x, y)
    res = run_bass_kernel_spmd(nc, in_maps, core_ids=list(range(NCORES)),
                               **spmd_kwargs)
    return _gather(res.results), res


def kernel(x, y):
    out, _ = run(x, y)
    return out



# revision 5
# speedup vs baseline: 1.0180x; 1.0180x over previous
"""Distributed Trainium2 kernel for quantized-mixed int8 matmul dequant.

Reference computation (M = K = N = 4096):
    xf = (x - X_ZP) * X_SCALE      # x int32 values in [-128, 127]
    yf = (y - Y_ZP) * Y_SCALE      # y int32 values in [0, 255]
    out = xf @ yf                  # float32 [M, N]

Strategy: 2D-shard the GEMM over 8 NeuronCores as a 2x4 grid
(M split 2 ways, N split 4 ways -> per-core C tile of 2048 x 1024),
with the matmul run in fp8 (E4M3) DoubleRow mode (double-pumped PE:
two k-rows per cell per pass, 2x bf16 matmul throughput; measured
216ns per [256k x 128m x 512n] matmul = the fp8 roofline, 110.6us
of matmul work per core).

fp8 precision scheme (rel err ~7e-3 vs the 2e-2 gate):
  x is centered:  x~ = (x + 0.5) * sqrt(S)   in [-127.5, 127.5]*sqrt(S)
  y is shifted:   y~ = (y - 160) * sqrt(S)   in [-160, 95]*sqrt(S)
  out[m,n] = sum_k x~ y~ + g[n],  g[n] = 65.5 * S * colsum(y - 160)[n]
The sqrt(S) prescale (S = X_SCALE*Y_SCALE) keeps fp8 relative precision
identical while making PSUM hold final-scale values, so the epilogue is
a single tensor_add of the exact (host-computed, fp32) g correction.
Centering x halves its top-end quantization step (128..193 would round
at step 16; +-127.5 rounds at step 8). fp8 e4m3 products are exact in
the PE's e10m10 intermediate, so a host numpy simulation of the fp8
rounding predicts the HW result bit-for-bit (verified: 6.991e-03 both).

Both fp8 operand shards are SBUF-resident (x 8MB + y 4MB of ~26MB), so
after the upload ramp the PE runs with zero DMA waits. Upload layout is
chosen so the ramp critical path is short: the first m-tile sweep needs
ALL of y but only the first m-half of x, so y is the SMALL shard (4MB,
~20us on one ring) and x's two m-halves upload in order on the other
ring. Each ring carries one sequential HBM stream — interleaving x/y
chunks across both rings makes 4 concurrent HBM streams, halves upload
bandwidth, stalls the PE >5us, and the HAM clock gate then runs the PE
~1.2x slow for the WHOLE kernel (216 -> 259ns per matmul, measured).
Per-double-chunk DMAs (256KB) keep any individual PE wait well under
the ~3.4us HAM window during the ramp.

Loop order is m-tile outer, k inner, n-group innermost: each
stationary x-slice is reused across the 2 moving n-groups and
LDWEIGHTS (135ns, no FWL in DoubleRow mode) hides under the 216ns
matmuls.
"""

import numpy as np
import ml_dtypes

import concourse.bacc as bacc
import concourse.mybir as mybir
import concourse.tile as tile
from concourse.bass_utils import run_bass_kernel_spmd

M = K = N = 4096
X_SCALE, X_ZP = 0.03, -66
Y_SCALE, Y_ZP = 0.025, 160
S = X_SCALE * Y_SCALE
SQS = np.float32(np.sqrt(S))
CX = 65.5                 # x centering shift: x - X_ZP = (x + 0.5) + CX

NCORES = 8
MSPLIT, NSPLIT = 2, 4
MC = M // MSPLIT          # 2048 rows of C per core
NC = N // NSPLIT          # 1024 cols of C per core
P = 128                   # partitions
KC2 = K // (2 * P)        # 16 double-chunks (256 k-rows each)
MT = MC // P              # 16 m-tiles
MH = 2                    # x uploaded in MH m-halves (first sweep needs one)
MCH = MC // MH            # 1024 x-columns per half
NF = 512                  # matmul out free dim (one PSUM bank at fp32)
NG = NC // NF             # 2 n-groups
XQ = 1                    # k-double-chunks per x tile (256KB DMAs)
# y tile sizes in double-chunks (256KB units). The head stays uniform
# 256KB — every variation that batched or mixed the EARLY upload
# measured the PE at ~2.0GHz (259ns/matmul) for the whole kernel
# instead of 2.4GHz (216ns). Batching only the tail cuts trigger
# overhead where it is safe.
YSIZES = [1] * 10 + [2] * 3
YOFFS = np.cumsum([0] + YSIZES[:-1]).tolist()
assert sum(YSIZES) == KC2

FP8 = mybir.dt.float8e4
E4NP = ml_dtypes.float8_e4m3

WARM_MM = 12              # 12 dummy warmup matmuls held 2.4GHz in 15/15
                          # runs; 8 lost the clock lottery in 4/8 runs
WARM_ROWS = 256           # moving rows per warmup matmul (512 spans ~5.2us
                          # at the 1.2GHz cold clock and overshoots the
                          # ~10.8us data-ready point; 256 ends right at it)
RAMP_MT = 4               # m-tiles interleaved during the upload ramp:
                          # each (y,x) 256KB chunk pair then feeds
                          # RAMP_MT*NG matmuls (~1.7us of PE work) vs
                          # ~1.4us/pair arrival, so the ramp is PE-bound
                          # instead of stalling ~4us on chunk waits
LAST_SPLIT = True

_CACHE = {}


def _build():
    nc = bacc.Bacc("TRN2", target_bir_lowering=False, debug=False)
    xt = nc.dram_tensor("xt", [MH, KC2 // XQ, P, 2 * XQ, MCH], FP8,
                        kind="ExternalInput")
    yts = [nc.dram_tensor(f"y{q}", [P, 2 * sz, NC], FP8, kind="ExternalInput")
           for q, sz in enumerate(YSIZES)]
    g = nc.dram_tensor("g", [P, NC], mybir.dt.float32, kind="ExternalInput")
    out = nc.dram_tensor("out", [MT, NG, P, NF], mybir.dt.float32,
                         kind="ExternalOutput")

    with tile.TileContext(nc) as tc:
        with (
            tc.tile_pool(name="warm_pool", bufs=1) as warm_pool,
            tc.tile_pool(name="xb_pool", bufs=MH * KC2 // XQ) as xb_pool,
            tc.tile_pool(name="yb_pool", bufs=len(YSIZES)) as yb_pool,
            tc.tile_pool(name="g_pool", bufs=1) as g_pool,
            tc.tile_pool(name="ot_pool", bufs=8) as ot_pool,
            tc.tile_pool(name="ps_pool", bufs=8, space="PSUM") as ps_pool,
        ):
            # PE warm-up: the upload ramp is DMA-heavy with little matmul
            # work; burn dummy matmuls so the HAM clock gate un-throttles
            # before the real matmuls issue.
            wt = warm_pool.tile([P, NF], mybir.dt.bfloat16, tag="wt")
            nc.vector.memset(wt[:], 0.0)
            wps = ps_pool.tile([64, NF], mybir.dt.float32, tag="ps", name="wps")
            for _ in range(WARM_MM):
                nc.tensor.matmul(wps[:, :WARM_ROWS], wt[:, :64],
                                 wt[:, :WARM_ROWS], start=True, stop=True)

            gt = g_pool.tile([P, NC], mybir.dt.float32, tag="gt")
            nc.gpsimd.dma_start(out=gt[:], in_=g[:, :])
            xb = [[xb_pool.tile([P, 2 * XQ, MCH], FP8, tag="xb",
                                name=f"xb{h}_{q}")
                   for q in range(KC2 // XQ)] for h in range(MH)]
            yb = [yb_pool.tile([P, 2 * sz, NC], FP8, tag="yb", name=f"yb{q}")
                  for q, sz in enumerate(YSIZES)]
            for q in range(len(YSIZES)):
                nc.scalar.dma_start(out=yb[q][:], in_=yts[q][:, :, :])
            for h in range(MH):
                for q in range(KC2 // XQ):
                    nc.sync.dma_start(out=xb[h][q][:], in_=xt[h, q, :, :, :])
            # k -> (y tile, chunk offset within tile)
            ymap = {}
            for q, (sz, off) in enumerate(zip(YSIZES, YOFFS)):
                for c in range(sz):
                    ymap[off + c] = (q, c)

            out_dma_engines = [nc.sync, nc.scalar]
            nout = [0]

            def drain(m, gi, psum):
                ot = ot_pool.tile([P, NF], mybir.dt.float32, tag="ot",
                                  name=f"ot{m}_{gi}")
                nc.vector.tensor_add(ot[:], psum[:],
                                     gt[:, gi * NF:(gi + 1) * NF])
                eng = out_dma_engines[nout[0] % 2]
                nout[0] += 1
                eng.dma_start(out=out[m, gi, :, :], in_=ot[:])

            def mm_sweep(m, groups, psums):
                h, mm = divmod(m, MT // MH)
                for k in range(KC2):
                    q, c = divmod(k, XQ)
                    xs = xb[h][q][:, 2 * c:2 * c + 2, mm * P:(mm + 1) * P]
                    yq, yc = ymap[k]
                    for j, gi in enumerate(groups):
                        nc.tensor.matmul(
                            psums[j][:], xs,
                            yb[yq][:, 2 * yc:2 * yc + 2,
                                   gi * NF:(gi + 1) * NF],
                            start=(k == 0), stop=(k == KC2 - 1),
                            perf_mode=mybir.MatmulPerfMode.DoubleRow)

            def ps_tiles(m, groups):
                return [ps_pool.tile([P, NF], mybir.dt.float32,
                                     tag="ps", name=f"ps{m}_{gi}")
                        for gi in groups]

            # Ramp pass: m-tiles 0..RAMP_MT-1 interleaved over k (k outer)
            # so each arriving (y[k], x[k]) chunk pair feeds RAMP_MT*NG
            # matmuls before the next pair is needed. Uses all 8 PSUM
            # banks; the drains free banks ~1us before the first
            # sequential sweep (m=RAMP_MT) needs them.
            ramp_ms = list(range(RAMP_MT))
            ramp_ps = {m: ps_tiles(m, list(range(NG))) for m in ramp_ms}
            for k in range(KC2):
                q, c = divmod(k, XQ)
                yq, yc = ymap[k]
                for m in ramp_ms:
                    h, mm = divmod(m, MT // MH)
                    xs = xb[h][q][:, 2 * c:2 * c + 2, mm * P:(mm + 1) * P]
                    for gi in range(NG):
                        nc.tensor.matmul(
                            ramp_ps[m][gi][:], xs,
                            yb[yq][:, 2 * yc:2 * yc + 2,
                                   gi * NF:(gi + 1) * NF],
                            start=(k == 0), stop=(k == KC2 - 1),
                            perf_mode=mybir.MatmulPerfMode.DoubleRow)
            for m in ramp_ms:
                for gi in range(NG):
                    drain(m, gi, ramp_ps[m][gi])

            n_plain = MT - 1 if LAST_SPLIT else MT
            for m in range(RAMP_MT, n_plain):
                groups = list(range(NG))
                psums = ps_tiles(m, groups)
                mm_sweep(m, groups, psums)
                for gi in groups:
                    drain(m, gi, psums[gi])

            if LAST_SPLIT:
                m = MT - 1
                # g0: full sweep + normal drain (overlaps g1's sweep)
                psums = ps_tiles(m, [0])
                mm_sweep(m, [0], psums)
                drain(m, 0, psums[0])
                # g1: final sweep; split its drain into two [P, NF/2]
                # halves — two short vector adds (gpsimd can't read
                # PSUM), out DMAs in parallel on both rings — to
                # shorten the post-last-matmul tail.
                psums = ps_tiles(m, [1])
                mm_sweep(m, [1], psums)
                HF = NF // 2
                add_engines = [nc.vector, nc.vector]
                for half in range(2):
                    ot = ot_pool.tile([P, NF], mybir.dt.float32, tag="ot",
                                      name=f"otf{half}")
                    lo = NF + half * HF
                    add_engines[half].tensor_add(
                        ot[:, :HF],
                        psums[0][:, half * HF:(half + 1) * HF],
                        gt[:, lo:lo + HF])
                    eng = out_dma_engines[(nout[0] + half) % 2]
                    eng.dma_start(
                        out=out[m, 1, :, half * HF:(half + 1) * HF],
                        in_=ot[:, :HF])
    nc.compile()
    return nc


def _get_nc():
    if "nc" not in _CACHE:
        _CACHE["nc"] = _build()
    return _CACHE["nc"]


def _chunk_block(a2d, ncols):
    """[K, ncols] -> [KC2, P, 2, ncols] with (c, p, i) -> k = 256c + 128i + p
    (the DoubleRow pairing)."""
    return np.ascontiguousarray(
        a2d.reshape(KC2, 2, P, ncols).transpose(0, 2, 1, 3))


def _shard(x, y):
    x = np.asarray(x, dtype=np.int32)
    y = np.asarray(y, dtype=np.int32)
    xq = ((x.astype(np.float32) + np.float32(0.5)) * SQS).astype(E4NP)
    yq = ((y.astype(np.float32) - np.float32(160.0)) * SQS).astype(E4NP)
    # exact column correction for the x centering shift
    gfull = (CX * S) * (y.astype(np.float64).sum(axis=0) - 160.0 * K)
    gfull = gfull.astype(np.float32)

    xts = []
    for mi in range(MSPLIT):
        blk = _chunk_block(
            np.ascontiguousarray(xq[mi * MC:(mi + 1) * MC, :].T), MC)
        # m-halves + quad-chunk tiles: [MH, KC2/XQ, P, 2*XQ, MCH]
        xts.append(np.ascontiguousarray(
            blk.reshape(KC2 // XQ, XQ, P, 2, MH, MCH)
            .transpose(4, 0, 2, 1, 3, 5)
            .reshape(MH, KC2 // XQ, P, 2 * XQ, MCH)))
    ys = []
    for ni in range(NSPLIT):
        blk = _chunk_block(
            np.ascontiguousarray(yq[:, ni * NC:(ni + 1) * NC]), NC)
        tiles = {}
        for q, (sz, off) in enumerate(zip(YSIZES, YOFFS)):
            # [sz, P, 2, NC] -> [P, 2*sz, NC]
            tiles[f"y{q}"] = np.ascontiguousarray(
                blk[off:off + sz].transpose(1, 0, 2, 3)
                .reshape(P, 2 * sz, NC))
        ys.append(tiles)
    gs = [np.ascontiguousarray(
              np.broadcast_to(gfull[ni * NC:(ni + 1) * NC], (P, NC)))
          for ni in range(NSPLIT)]
    in_maps = []
    for c in range(NCORES):
        mi, ni = divmod(c, NSPLIT)
        in_maps.append({"xt": xts[mi], "g": gs[ni], **ys[ni]})
    return in_maps


def _gather(results):
    out = np.empty((M, N), dtype=np.float32)
    for c in range(NCORES):
        mi, ni = divmod(c, NSPLIT)
        blk = results[c]["out"]  # [MT, NG, P, NF]
        out[mi * MC:(mi + 1) * MC, ni * NC:(ni + 1) * NC] = \
            blk.transpose(0, 2, 1, 3).reshape(MC, NC)
    return out


def run(x, y, **spmd_kwargs):
    """Run and return (full_output, BassKernelResults)."""
    nc = _get_nc()
    in_maps = _shard(x, y)
    res = run_bass_kernel_spmd(nc, in_maps, core_ids=list(range(NCORES)),
                               **spmd_kwargs)
    return _gather(res.results), res


def kernel(x, y):
    out, _ = run(x, y)
    return out

